# revision 101
# baseline (speedup 1.0000x reference)
"""Multi-head causal attention on 8 Trainium2 NeuronCores (Bass/Tile).

Problem: B=4, S=1024, D=1024, H=16 heads (dk=64), causal mask, fp32 I/O.

Sharding: 8 cores = 4 batches x 2 head-groups (8 heads each).
  Wq/Wk/Wv sharded column-wise by head (tensor parallel), Wo row-wise;
  the Wo all-reduce is a host-side pairwise sum (2 cores per batch).

Per-core kernel (bf16 matmul operands, fp32 PSUM accumulate):
  phase P: Q^T zero-padded per head into qtz (slot h holds Q_h^T on its
           64 partitions, zeros elsewhere, so score matmuls contract
           K=128 at full rate without mixing heads), K^T packed
           [128, 4, S].
  phase A: V projection ([s, d'] orientation) interleaved with the
           score/attnV stream: v-tiles 0-1, the qj=0 steps, v-tiles 2-3,
           then qj=1.  Per step: scores^T (causally width-trimmed), exp
           on ACT (no max subtraction: |scores/8| < ~6), 0/1 mask
           multiply on the diagonal 128-block only, attnV accumulated
           with a 65th ones column of V as the softmax denominator.
           Denominator rows are DMA-gathered into den8.
  phase O: reciprocal, selector-matmul broadcast of 1/den over partition
           halves, in-place normalize of headout^T, output projection,
           bf16 store (host gather upcasts and adds bias).

Scheduling (the structural wins over the naive phase loop):
  - All pools and long-lived tiles are created once; constants (mask,
    sel8, ones columns, qtz zero padding) and all weight loads live in a
    prologue outside the repeat loop.
  - PSUM plan fits 8 banks with no pool-boundary barriers: pool1 =
    2x [128,2,512] (V-proj / score tiles ONLY), pool2 = 4x [128,512]
    (Q/K proj tiles, attnV accumulators, 1/den broadcast, out-proj
    accumulators).  Keeping out-proj tiles out of pool1 matters: a
    shared tag couples the out-proj rotation to the score rotation and
    strangles the score/exp pipeline.
  - DMA rings: activation loads for the next iteration are issued from
    the SP ring between the den8 gathers and the out stores, so their
    transfers stream behind compute and out stores may lag into the next
    iteration (they are only read at run end).  Gathers/stores stay on
    the SP ring; a DMA whose producer finishes late must never sit in a
    busy sequencer's stream (it stalls that engine's whole queue).
  - tc.no_sync_barrier() between the K projection and phase A stops the
    tile scheduler from hoisting V-proj matmuls (whose xv load lands
    last) above the K stream, which would block the in-order PE.
  - The repeat loop body is 4 unrolled iterations per For_i trip,
    because For_i drains all engines at its back edge.

fp8 note: e4m3 DoubleRow matmuls (with host-side hi/lo compensation,
which reaches rel err 3.1e-3 vs bf16's 4.7e-3) were tried for the
projections and measured on hardware at 1.0 cycles/row -- the 2x
double-pumping in the cost model does not materialize through this
codegen path, so the 1.5x instruction count makes them ~50% slower
than bf16.  Projections stay bf16.

ROOFLINE (isolated matmul microbenches, mb2.py): the PE on this
hardware sustains 512-col bf16 matmuls at 292.6 ns = 1.75 GHz effective
(not the nominal 2.4 GHz); half-width matmuls scale exactly (147.6 ns),
so it is a rate limit, not per-instruction overhead.  Plain fp8: 270 ns
(no byte-bandwidth win).  209k PE-cycles/iter / 1.75 GHz = 119 us --
the measured 110-118 us IS the practical roofline.

Measured dead-ends (HW, (T(400)-T(1))/399; +-4us run noise plus ~10us
session-scale apparatus drift -- always A/B against an adjacent
baseline run):
  - UNROLL 8: +11us (iram), UNROLL 2: +14us (For_i drain); 4 is best.
  - For_i(staggered_reset=True) (removes the back-edge all-engine
    barrier/reset block): correct, but HW +7us -- the clean per-trip
    drain beats in-body distributed semaphore resets here.
  - Pipelining emit_O(1) into the next P phase: sim -5us, HW +5us.
  - 256-wide halfstep score tiles + 3-deep lookahead + per-pair
    normalization: sim 95.5us (vs 101.1) but HW 126-129us.
  - 64-granular diagonal score trim (splits each diag score matmul by
    key halves, saving 4096 cycles): numerically exact, but HW +6us in
    an adjacent A/B -- the (0,64) tile_position matmuls cost more than
    the cycles saved (PE tiling reconfiguration?).
  - Replacing the 8 sel8 matmuls with free-dim-replicated broadcast
    DMAs (in_=rec8[r:r+1,None,qsl].broadcast_to([1,64,512]) -- works
    and is exact): HW +50us (!); 16 x 64-descriptor replicated reads
    per iter swamp the DMA path.  GpSimd InstPartitionBroadcast would
    also do it but its Q7 ucode lib fails to compile via bass2jax.
  - PSUM hazard: a second start=True matmul to a bank with an OPEN
    accumulation group corrupts the open group's region; closed
    start+stop groups on one bank are fine.
  - bf16 sel8/rec8 normalization path (hypothesis: f32r moving runs
    slower than bf16 on this HW): 116.2us vs 116.0us adjacent baseline
    -- identical; f32r moving is NOT slower.  Reverted to keep the
    f32r path's better error margin (4.8e-3 vs 6.0e-3).
  - Early x8 refresh (q/k before the no_sync_barrier, v after
    v_tpair(3)) to get the ~2MB of transfers off the For_i back-edge
    drain: emitted AFTER the barrier it races (rel 2.6e-1 cold-start;
    no_sync edges carry no semaphores, so the WAR on x8 is order-only
    -- confirmed in tile.py) and times 115.8us; emitted BEFORE the
    barrier it is correct (3x cold-start validated) but times 129.6us
    -- the synthesized WAR semaphores on the P matmuls cost more than
    the drain saving.  Keep the late refresh.
"""

from contextlib import ExitStack

import ml_dtypes
import numpy as np

import concourse.bacc as bacc
import concourse.tile as tile
from concourse import mybir
from concourse.bass_utils import run_bass_kernel_spmd

F32R = mybir.dt.float32r
F32 = mybir.dt.float32
BF16 = mybir.dt.bfloat16
EXP = mybir.ActivationFunctionType.Exp

S = 1024  # sequence length
D = 1024  # model dim
DK = 64  # head dim
HPC = 8  # heads per core
N_CORES = 8
UNROLL = 4  # loop-body unroll (For_i drains all engines at its back edge)
EXPSCALE = 1.0 / np.sqrt(DK)  # folded into the exp activation


def _alloc_persistent(nc, tc, ctx, t):
    """Create pools + long-lived tiles; emit the once-only prologue."""
    pl = {
        "main": ctx.enter_context(tc.tile_pool(name="main", bufs=1)),
        "pool1": ctx.enter_context(
            tc.tile_pool(name="pool1", bufs=2, space="PSUM")
        ),
        "pool2": ctx.enter_context(
            tc.tile_pool(name="pool2", bufs=4, space="PSUM")
        ),
        "epool": ctx.enter_context(tc.tile_pool(name="epool", bufs=12)),
        "xtr": ctx.enter_context(tc.tile_pool(name="xtr", bufs=6)),
        "osb": ctx.enter_context(tc.tile_pool(name="osb", bufs=4)),
    }
    main = pl["main"]
    shapes = {
        "qtz": ([128, 8, S], BF16),
        "kt_sb": ([128, 4, S], BF16),
        "v_sb": ([128, 8, 8, 65], BF16),  # (kpos, ki, head, d+1)
        "hout_sb": ([128, 4, S], BF16),
        "maskd": ([128, 128], BF16),
        "sel8": ([8, 512], F32R),
        "den8": ([8, S], F32),
        "rec8": ([8, S], F32R),
        "wo_sb": ([128, 4, S], BF16),
        "x8q": ([128, 8, S], BF16),
        "x8k": ([128, 8, S], BF16),
        "x8v": ([128, 8, S], BF16),
        "w8q": ([128, 8, 512], BF16),
        "w8k": ([128, 8, 512], BF16),
        "w8v": ([128, 8, 512], BF16),
    }
    tl = {k: main.tile(shp, dt, name=k) for k, (shp, dt) in shapes.items()}
    # prologue (loop-invariant): constants, ones columns, qtz zero padding,
    # and the weight loads (weights-resident steady state; the repeat-1
    # correctness path pays them once like any other load)
    nc.sync.dma_start(out=tl["sel8"], in_=t["sel8"][:, :])
    nc.sync.dma_start(out=tl["maskd"], in_=t["maskd"][:, :])
    for which in ("q", "k", "v"):
        nc.scalar.dma_start(
            out=tl["w8" + which].rearrange("p a b -> p (a b)"),
            in_=t["w" + which + "_t"][:, :],
        )
        nc.scalar.dma_start(
            out=tl["x8" + which],
            in_=t["x" + which + "_t"].rearrange("(n p) s -> p n s", p=128),
        )
    nc.scalar.dma_start(
        out=tl["wo_sb"].rearrange("p a b -> p (a b)"), in_=t["wo_s"][:, :]
    )
    nc.vector.memset(tl["v_sb"][:, :, :, 64:65], 1.0)
    nc.vector.memset(tl["qtz"].rearrange("p a b -> p (a b)"), 0.0)
    return pl, tl


def _emit_iter(nc, tc, t, tl, pl):
    """One forward pass (assumes prologue already emitted)."""
    pool1, pool2 = pl["pool1"], pl["pool2"]
    epool, xtr, osb = pl["epool"], pl["xtr"], pl["osb"]
    qtz, kt_sb, v_sb = tl["qtz"], tl["kt_sb"], tl["v_sb"]
    hout_sb, maskd, sel8 = tl["hout_sb"], tl["maskd"], tl["sel8"]
    den8, rec8, wo_sb = tl["den8"], tl["rec8"], tl["wo_sb"]



    def proj_tile(ps_ap, w8, x8, wsl, xsl, swap=False):
        """Accumulate the 8-chunk bf16 product into ps_ap."""
        for c in range(8):
            lhs, rhs = w8[:, c, wsl], x8[:, c, xsl]
            if swap:
                lhs, rhs = rhs, lhs
            nc.tensor.matmul(ps_ap, lhs, rhs, start=(c == 0), stop=(c == 7))

    # ================= phase P: Q, K projections =================
    for which in ("q", "k"):
        x8, w8 = tl["x8" + which], tl["w8" + which]
        for sj in range(2):
            sjs = slice(512 * sj, 512 * (sj + 1))
            for dtile in range(4):
                ps = pool2.tile([128, 512], F32, tag="ps")
                proj_tile(
                    ps, w8, x8,
                    slice(128 * dtile, 128 * (dtile + 1)), sjs,
                )
                if which == "q":
                    nc.scalar.copy(qtz[0:64, 2 * dtile, sjs], ps[0:64, :])
                    nc.scalar.copy(
                        qtz[64:128, 2 * dtile + 1, sjs], ps[64:128, :]
                    )
                else:
                    nc.scalar.copy(kt_sb[:, dtile, sjs], ps)

    tc.no_sync_barrier()

    # ======== phase A: V projection interleaved with attention ========
    def v_tpair(tp):
        ps2 = pool1.tile([128, 2, 512], F32, tag="sc")
        for sub in range(2):
            stile = 2 * tp + sub
            proj_tile(
                ps2[:, sub, :], tl["w8v"], tl["x8v"],
                slice(0, 512), slice(128 * stile, 128 * (stile + 1)),
                swap=True,
            )
            nc.vector.tensor_copy(
                v_sb[:, stile, :, 0:64],
                ps2[:, sub, :].rearrange("p (h c) -> p h c", c=64),
            )

    def emit_score(qj, hc, ki):
        b = 128 * max(0, ki - 4 * qj)
        kis = slice(128 * ki, 128 * (ki + 1))
        sc = pool1.tile([128, 2, 512], F32, tag="sc")
        for sub in range(2):
            nc.tensor.matmul(
                sc[:, sub, b:512],
                kt_sb[:, hc, kis],
                qtz[:, 2 * hc + sub, 512 * qj + b : 512 * (qj + 1)],
                start=True,
                stop=True,
            )
        return sc

    def emit_O(qj):
        """Normalize + output projection + store for one q-half."""
        qsl = slice(512 * qj, 512 * (qj + 1))
        with nc.allow_low_precision(reason="softmax reciprocal"):
            nc.vector.reciprocal(rec8[:, qsl], den8[:, qsl])
        for hc in range(4):
            bp = pool2.tile([128, 512], F32, tag="ps")
            nc.tensor.matmul(
                bp,
                sel8[:, 128 * hc : 128 * (hc + 1)],
                rec8[:, qsl],
                start=True,
                stop=True,
            )
            nc.vector.tensor_mul(hout_sb[:, hc, qsl], hout_sb[:, hc, qsl], bp)
        for stile in range(4 * qj, 4 * qj + 4):
            out_sb = osb.tile([128, S], BF16, tag="out")
            for ej in range(2):
                op1 = pool2.tile([128, 512], F32, tag="ps")
                for hc in range(4):
                    nc.tensor.matmul(
                        op1,
                        hout_sb[:, hc, 128 * stile : 128 * (stile + 1)],
                        wo_sb[:, hc, 512 * ej : 512 * (ej + 1)],
                        start=(hc == 0),
                        stop=(hc == 3),
                    )
                esl = slice(512 * ej, 512 * (ej + 1))
                nc.scalar.copy(out_sb[:, esl], op1)
                nc.sync.dma_start(
                    out=t["out_p"][128 * stile : 128 * (stile + 1), esl],
                    in_=out_sb[:, esl],
                )

    steps = []
    for qj in range(2):
        kmax = 4 if qj == 0 else 8
        for hc in range(4):
            for ki in range(kmax):
                steps.append((qj, hc, ki, kmax))
    n_qj0 = 16

    def emit_exp(qj, hc, ki, sc):
        """exp (+ diagonal mask) for one step; returns the ee tile."""
        b = 128 * max(0, ki - 4 * qj)
        ee = epool.tile([128, 2, 512], BF16, tag="e", name="ee")
        nc.scalar.activation(
            ee[:, :, b:512], sc[:, :, b:512], EXP, scale=float(EXPSCALE)
        )
        if ki - 4 * qj >= 0:  # diagonal block: 0/1 mask, both heads at once
            nc.vector.tensor_mul(
                ee[:, :, b : b + 128],
                ee[:, :, b : b + 128],
                maskd[:, None, :].broadcast_to([128, 2, 128]),
            )
        return ee

    v_tpair(0)
    v_tpair(1)
    sc_next = emit_score(*steps[0][:3])
    avs = {}
    for i, (qj, hc, ki, kmax) in enumerate(steps):
        qsl = slice(512 * qj, 512 * (qj + 1))
        if ki == 0:
            av_e = pool2.tile([128, 512], F32, tag="ps")
            av_o = pool2.tile([128, 512], F32, tag="ps")
            avs[(qj, hc)] = (av_e, av_o)
        o_e, o_o = avs[(qj, hc)]
        sc = sc_next
        if i + 1 == n_qj0:  # qj=0 done being scored: V stiles 4-7 next
            v_tpair(2)
            v_tpair(3)
        if i + 1 < len(steps):  # cross-pair score lookahead
            sc_next = emit_score(*steps[i + 1][:3])
        b = 128 * max(0, ki - 4 * qj)
        ee = emit_exp(qj, hc, ki, sc)
        nc.tensor.matmul(
            o_e[0:65, b:512],
            v_sb[:, ki, 2 * hc, :],
            ee[:, 0, b:512],
            start=(ki == 0),
            stop=(ki == kmax - 1),
        )
        nc.tensor.matmul(
            o_o[0:65, b:512],
            v_sb[:, ki, 2 * hc + 1, :],
            ee[:, 1, b:512],
            start=(ki == 0),
            stop=(ki == kmax - 1),
        )
        if ki != kmax - 1:
            continue
        del avs[(qj, hc)]
        # denominator rows first: they unblock the reciprocal -> bp matmul
        # chain in phase O, so they must not queue behind the big copies
        de_t = xtr.tile([1, 512], F32, tag="de")
        do_t = xtr.tile([1, 512], F32, tag="do")
        nc.vector.tensor_copy(de_t, o_e[64:65, :])
        nc.vector.tensor_copy(do_t, o_o[64:65, :])
        nc.sync.dma_start(out=den8[2 * hc : 2 * hc + 1, qsl], in_=de_t)
        nc.sync.dma_start(out=den8[2 * hc + 1 : 2 * hc + 2, qsl], in_=do_t)
        # extract headout^T
        nc.vector.tensor_copy(hout_sb[0:64, hc, qsl], o_e[0:64, :])
        otmp = xtr.tile([64, 512], BF16, tag="otmp")
        nc.vector.tensor_copy(otmp, o_o[0:64, :])
        nc.sync.dma_start(out=hout_sb[64:128, hc, qsl], in_=otmp)

    # next-iteration activation refresh on the SP ring: issues after the
    # den8 gathers, ahead of the out_p stores (which may lag into the next
    # iteration's P phase -- they are only read at run end)
    for which in ("q", "k", "v"):
        xdr = t["x" + which + "_t"].rearrange("(n p) s -> p n s", p=128)
        for half in range(2):
            hs = slice(512 * half, 512 * (half + 1))
            nc.sync.dma_start(
                out=tl["x8" + which][:, :, hs], in_=xdr[:, :, hs]
            )

    emit_O(0)
    emit_O(1)


def _build(repeat=1):
    nc = bacc.Bacc()
    t = {}
    for name in ("xq_t", "xk_t", "xv_t"):
        t[name] = nc.dram_tensor(name, [D, S], BF16, kind="ExternalInput")
    for name in ("wq_t", "wk_t", "wv_t"):
        t[name] = nc.dram_tensor(name, [128, 8 * 512], BF16, kind="ExternalInput")
    t["wo_s"] = nc.dram_tensor("wo_s", [128, 4 * D], BF16, kind="ExternalInput")
    t["maskd"] = nc.dram_tensor("maskd", [128, 128], BF16, kind="ExternalInput")
    t["sel8"] = nc.dram_tensor("sel8", [8, 512], F32R, kind="ExternalInput")
    t["out_p"] = nc.dram_tensor("out_p", [S, D], BF16, kind="ExternalOutput")

    with tile.TileContext(nc) as tc:
        with ExitStack() as ctx:
            pl, tl = _alloc_persistent(nc, tc, ctx, t)
            n_loop, rem = divmod(repeat, UNROLL)
            if n_loop > 0:
                with tc.For_i(0, n_loop, 1):
                    for _ in range(UNROLL):
                        _emit_iter(nc, tc, t, tl, pl)
            for _ in range(rem):
                _emit_iter(nc, tc, t, tl, pl)
    nc.compile()
    return nc


_CACHE = {}


def _get(repeat=1):
    if repeat not in _CACHE:
        _CACHE[repeat] = _build(repeat)
    return _CACHE[repeat]


def _host_prep(query, key, value, mask, Wq, Wk, Wv, Wo):
    """Build the per-core in_maps. Returns None if mask isn't causal tril."""
    m = np.asarray(mask)[0, 0]
    if not np.array_equal(m, np.tril(np.ones((S, S), m.dtype))):
        return None

    bf = ml_dtypes.bfloat16

    # diagonal-block mask (same for every diagonal tile under causal tril)
    maskd = m[0:128, 0:128].T.astype(bf)

    sel8 = np.zeros((8, 512), np.float32)
    for hc in range(4):
        sel8[2 * hc, 128 * hc : 128 * hc + 64] = 1.0
        sel8[2 * hc + 1, 128 * hc + 64 : 128 * hc + 128] = 1.0

    def ileave(a):  # [R, C] -> [128, (R//128)*C]: chunk-c data contiguous per p
        R, C = a.shape
        return np.ascontiguousarray(
            a.reshape(R // 128, 128, C).transpose(1, 0, 2).reshape(128, -1)
        )

    in_maps = []
    for c in range(N_CORES):
        b, g = c // 2, c % 2
        gsl = slice(512 * g, 512 * (g + 1))
        in_maps.append(
            {
                "xq_t": np.ascontiguousarray(query[b].T.astype(bf)),
                "xk_t": np.ascontiguousarray(key[b].T.astype(bf)),
                "xv_t": np.ascontiguousarray(value[b].T.astype(bf)),
                "wq_t": ileave(Wq[gsl, :].T.astype(bf)),
                "wk_t": ileave(Wk[gsl, :].T.astype(bf)),
                "wv_t": ileave(Wv[gsl, :].T.astype(bf)),
                "wo_s": ileave(Wo[:, gsl].T.astype(bf)),
                "maskd": maskd,
                "sel8": sel8,
            }
        )
    return in_maps


def _gather(results, bo, B):
    out = np.empty((B, S, D), np.float32)
    for b in range(B):
        out[b] = (
            results[2 * b]["out_p"].astype(np.float32)
            + results[2 * b + 1]["out_p"].astype(np.float32)
            + np.asarray(bo)[None, :]
        )
    return out


def _reference_fallback(query, key, value, mask, Wq, Wk, Wv, Wo, bo):
    B = query.shape[0]
    H = 16
    dk = D // H
    q = np.asarray(query, np.float32)
    k = np.asarray(key, np.float32)
    v = np.asarray(value, np.float32)

    def proj(x, W):
        return (x @ W.T).reshape(B, S, H, dk).transpose(0, 2, 1, 3)

    Q, K, V = proj(q, Wq), proj(k, Wk), proj(v, Wv)
    sc = np.einsum("bhqd,bhkd->bhqk", Q, K) / np.sqrt(np.float32(dk))
    sc = np.where(np.asarray(mask) == 0, np.float32(-1e9), sc)
    sc = sc - sc.max(axis=-1, keepdims=True)
    a = np.exp(sc)
    a = a / a.sum(axis=-1, keepdims=True)
    o = np.einsum("bhqk,bhkd->bhqd", a, V).transpose(0, 2, 1, 3).reshape(B, S, D)
    return (o @ np.asarray(Wo).T + np.asarray(bo)).astype(np.float32)


def kernel(query, key, value, mask, Wq, Wk, Wv, Wo, bo):
    query = np.asarray(query, np.float32)
    key = np.asarray(key, np.float32)
    value = np.asarray(value, np.float32)
    Wq, Wk, Wv, Wo = (np.asarray(w, np.float32) for w in (Wq, Wk, Wv, Wo))
    in_maps = _host_prep(query, key, value, mask, Wq, Wk, Wv, Wo)
    if in_maps is None:  # non-causal mask: host fallback
        return _reference_fallback(query, key, value, mask, Wq, Wk, Wv, Wo, bo)
    nc = _get(1)
    res = run_bass_kernel_spmd(nc, in_maps, list(range(N_CORES)))
    return _gather(res.results, bo, query.shape[0])


def run_spmd(in_maps, repeat=1):
    """For test.py: run prebuilt kernel, return BassKernelResults."""
    nc = _get(repeat)
    return run_bass_kernel_spmd(nc, in_maps, list(range(N_CORES)))


def host_prep(*args, **kw):
    return _host_prep(*args, **kw)


def gather(results, bo, B=4):
    return _gather(results, bo, B)



# revision 105
# speedup vs baseline: 1.0184x; 1.0184x over previous
"""Multi-head causal attention on 8 Trainium2 NeuronCores (Bass/Tile).

Problem: B=4, S=1024, D=1024, H=16 heads (dk=64), causal mask, fp32 I/O.

Sharding: 8 cores = 4 batches x 2 head-groups (8 heads each).
  Wq/Wk/Wv sharded column-wise by head (tensor parallel), Wo row-wise;
  the Wo all-reduce is a host-side pairwise sum (2 cores per batch).

Per-core kernel (bf16 matmul operands, fp32 PSUM accumulate):
  phase P: Q^T zero-padded per head into qtz (slot h holds Q_h^T on its
           64 partitions, zeros elsewhere, so score matmuls contract
           K=128 at full rate without mixing heads), K^T packed
           [128, 4, S].
  phase A: V projection ([s, d'] orientation) interleaved with the
           score/attnV stream: v-tiles 0-1, the qj=0 steps, v-tiles 2-3,
           then qj=1.  Per step: scores^T (causally width-trimmed), exp
           on ACT (no max subtraction: |scores/8| < ~6), 0/1 mask
           multiply on the diagonal 128-block only, attnV accumulated
           with a 65th ones column of V as the softmax denominator.
           Denominator rows are DMA-gathered into den8.
  phase O: reciprocal, selector-matmul broadcast of 1/den over partition
           halves, in-place normalize of headout^T, output projection,
           bf16 store (host gather upcasts and adds bias).

Scheduling (the structural wins over the naive phase loop):
  - All pools and long-lived tiles are created once; constants (mask,
    sel8, ones columns, qtz zero padding) and all weight loads live in a
    prologue outside the repeat loop.
  - PSUM plan fits 8 banks with no pool-boundary barriers: pool1 =
    2x [128,2,512] (V-proj / score tiles ONLY), pool2 = 4x [128,512]
    (Q/K proj tiles, attnV accumulators, 1/den broadcast, out-proj
    accumulators).  Keeping out-proj tiles out of pool1 matters: a
    shared tag couples the out-proj rotation to the score rotation and
    strangles the score/exp pipeline.
  - DMA rings: activation loads for the next iteration are issued from
    the SP ring between the den8 gathers and the out stores, so their
    transfers stream behind compute and out stores may lag into the next
    iteration (they are only read at run end).  Gathers/stores stay on
    the SP ring; a DMA whose producer finishes late must never sit in a
    busy sequencer's stream (it stalls that engine's whole queue).
  - tc.no_sync_barrier() between the K projection and phase A stops the
    tile scheduler from hoisting V-proj matmuls (whose xv load lands
    last) above the K stream, which would block the in-order PE.
  - The repeat loop body is 4 unrolled iterations per For_i trip,
    because For_i drains all engines at its back edge.

fp8 note: e4m3 DoubleRow matmuls (with host-side hi/lo compensation,
which reaches rel err 3.1e-3 vs bf16's 4.7e-3) were tried for the
projections and measured on hardware at 1.0 cycles/row -- the 2x
double-pumping in the cost model does not materialize through this
codegen path, so the 1.5x instruction count makes them ~50% slower
than bf16.  Projections stay bf16.

ROOFLINE (isolated matmul microbenches, mb2.py): the PE on this
hardware sustains 512-col bf16 matmuls at 292.6 ns = 1.75 GHz effective
(not the nominal 2.4 GHz); half-width matmuls scale exactly (147.6 ns),
so it is a rate limit, not per-instruction overhead.  Plain fp8: 270 ns
(no byte-bandwidth win).  209k PE-cycles/iter / 1.75 GHz = 119 us --
the measured 110-118 us IS the practical roofline.

Measured dead-ends (HW, (T(400)-T(1))/399; +-4us run noise plus ~10us
session-scale apparatus drift -- always A/B against an adjacent
baseline run):
  - UNROLL 8: +11us (iram), UNROLL 2: +14us (For_i drain); 4 is best.
  - For_i(staggered_reset=True) (removes the back-edge all-engine
    barrier/reset block): correct, but HW +7us -- the clean per-trip
    drain beats in-body distributed semaphore resets here.
  - Pipelining emit_O(1) into the next P phase: sim -5us, HW +5us.
  - 256-wide halfstep score tiles + 3-deep lookahead + per-pair
    normalization: sim 95.5us (vs 101.1) but HW 126-129us.
  - 64-granular diagonal score trim (splits each diag score matmul by
    key halves, saving 4096 cycles): numerically exact, but HW +6us in
    an adjacent A/B -- the (0,64) tile_position matmuls cost more than
    the cycles saved (PE tiling reconfiguration?).
  - Replacing the 8 sel8 matmuls with free-dim-replicated broadcast
    DMAs (in_=rec8[r:r+1,None,qsl].broadcast_to([1,64,512]) -- works
    and is exact): HW +50us (!); 16 x 64-descriptor replicated reads
    per iter swamp the DMA path.  GpSimd InstPartitionBroadcast would
    also do it but its Q7 ucode lib fails to compile via bass2jax.
  - PSUM hazard: a second start=True matmul to a bank with an OPEN
    accumulation group corrupts the open group's region; closed
    start+stop groups on one bank are fine.
  - bf16 sel8/rec8 normalization path (hypothesis: f32r moving runs
    slower than bf16 on this HW): 116.2us vs 116.0us adjacent baseline
    -- identical; f32r moving is NOT slower.  Reverted to keep the
    f32r path's better error margin (4.8e-3 vs 6.0e-3).
  - Early x8 refresh (q/k before the no_sync_barrier, v after
    v_tpair(3)) to get the ~2MB of transfers off the For_i back-edge
    drain: emitted AFTER the barrier it races (rel 2.6e-1 cold-start;
    no_sync edges carry no semaphores, so the WAR on x8 is order-only
    -- confirmed in tile.py) and times 115.8us; emitted BEFORE the
    barrier it is correct (3x cold-start validated) but times 129.6us
    -- the synthesized WAR semaphores on the P matmuls cost more than
    the drain saving.  Keep the late refresh.
"""

from contextlib import ExitStack

import ml_dtypes
import numpy as np

import concourse.bacc as bacc
import concourse.tile as tile
from concourse import mybir
from concourse.bass_utils import run_bass_kernel_spmd

F32R = mybir.dt.float32r
F32 = mybir.dt.float32
BF16 = mybir.dt.bfloat16
EXP = mybir.ActivationFunctionType.Exp

S = 1024  # sequence length
D = 1024  # model dim
DK = 64  # head dim
HPC = 8  # heads per core
N_CORES = 8
UNROLL = 4  # loop-body unroll (For_i drains all engines at its back edge)
EXPSCALE = 1.0 / np.sqrt(DK)  # folded into the exp activation


def _alloc_persistent(nc, tc, ctx, t):
    """Create pools + long-lived tiles; emit the once-only prologue."""
    pl = {
        "main": ctx.enter_context(tc.tile_pool(name="main", bufs=1)),
        "pool1": ctx.enter_context(
            tc.tile_pool(name="pool1", bufs=2, space="PSUM")
        ),
        "pool2": ctx.enter_context(
            tc.tile_pool(name="pool2", bufs=4, space="PSUM")
        ),
        "epool": ctx.enter_context(tc.tile_pool(name="epool", bufs=12)),
        "xtr": ctx.enter_context(tc.tile_pool(name="xtr", bufs=6)),
        "osb": ctx.enter_context(tc.tile_pool(name="osb", bufs=4)),
    }
    main = pl["main"]
    shapes = {
        "qtz": ([128, 8, S], BF16),
        "kt_sb": ([128, 4, S], BF16),
        "v_sb": ([128, 8, 8, 65], BF16),  # (kpos, ki, head, d+1)
        "hout_sb": ([128, 4, S], BF16),
        "maskd": ([128, 128], BF16),
        "sel8": ([8, 512], F32R),
        "den8": ([8, S], F32),
        "rec8": ([8, S], F32R),
        "wo_sb": ([128, 4, S], BF16),
        "x8q": ([128, 8, S], BF16),
        "x8k": ([128, 8, S], BF16),
        "x8v": ([128, 8, S], BF16),
        "w8q": ([128, 8, 512], BF16),
        "w8k": ([128, 8, 512], BF16),
        "w8v": ([128, 8, 512], BF16),
    }
    tl = {k: main.tile(shp, dt, name=k) for k, (shp, dt) in shapes.items()}
    # prologue (loop-invariant): constants, ones columns, qtz zero padding,
    # and the weight loads (weights-resident steady state; the repeat-1
    # correctness path pays them once like any other load)
    nc.sync.dma_start(out=tl["sel8"], in_=t["sel8"][:, :])
    nc.sync.dma_start(out=tl["maskd"], in_=t["maskd"][:, :])
    for which in ("q", "k", "v"):
        nc.scalar.dma_start(
            out=tl["w8" + which].rearrange("p a b -> p (a b)"),
            in_=t["w" + which + "_t"][:, :],
        )
        nc.scalar.dma_start(
            out=tl["x8" + which],
            in_=t["x" + which + "_t"].rearrange("(n p) s -> p n s", p=128),
        )
    nc.scalar.dma_start(
        out=tl["wo_sb"].rearrange("p a b -> p (a b)"), in_=t["wo_s"][:, :]
    )
    nc.vector.memset(tl["v_sb"][:, :, :, 64:65], 1.0)
    nc.vector.memset(tl["qtz"].rearrange("p a b -> p (a b)"), 0.0)
    return pl, tl


def _emit_iter(nc, tc, t, tl, pl):
    """One forward pass (assumes prologue already emitted)."""
    pool1, pool2 = pl["pool1"], pl["pool2"]
    epool, xtr, osb = pl["epool"], pl["xtr"], pl["osb"]
    qtz, kt_sb, v_sb = tl["qtz"], tl["kt_sb"], tl["v_sb"]
    hout_sb, maskd, sel8 = tl["hout_sb"], tl["maskd"], tl["sel8"]
    den8, rec8, wo_sb = tl["den8"], tl["rec8"], tl["wo_sb"]



    def proj_tile(ps_ap, w8, x8, wsl, xsl, swap=False):
        """Accumulate the 8-chunk bf16 product into ps_ap."""
        for c in range(8):
            lhs, rhs = w8[:, c, wsl], x8[:, c, xsl]
            if swap:
                lhs, rhs = rhs, lhs
            nc.tensor.matmul(ps_ap, lhs, rhs, start=(c == 0), stop=(c == 7))

    # ================= phase P: Q, K projections =================
    for which in ("q", "k"):
        x8, w8 = tl["x8" + which], tl["w8" + which]
        for sj in range(2):
            sjs = slice(512 * sj, 512 * (sj + 1))
            for dtile in range(4):
                ps = pool2.tile([128, 512], F32, tag="ps")
                proj_tile(
                    ps, w8, x8,
                    slice(128 * dtile, 128 * (dtile + 1)), sjs,
                )
                if which == "q":
                    nc.scalar.copy(qtz[0:64, 2 * dtile, sjs], ps[0:64, :])
                    nc.scalar.copy(
                        qtz[64:128, 2 * dtile + 1, sjs], ps[64:128, :]
                    )
                else:
                    nc.scalar.copy(kt_sb[:, dtile, sjs], ps)

    tc.no_sync_barrier()

    # ======== phase A: V projection interleaved with attention ========
    def v_tpair(tp):
        ps2 = pool1.tile([128, 2, 512], F32, tag="sc")
        for sub in range(2):
            stile = 2 * tp + sub
            proj_tile(
                ps2[:, sub, :], tl["w8v"], tl["x8v"],
                slice(0, 512), slice(128 * stile, 128 * (stile + 1)),
                swap=True,
            )
            nc.vector.tensor_copy(
                v_sb[:, stile, :, 0:64],
                ps2[:, sub, :].rearrange("p (h c) -> p h c", c=64),
            )

    def emit_score(qj, hc, ki):
        b = 128 * max(0, ki - 4 * qj)
        kis = slice(128 * ki, 128 * (ki + 1))
        sc = pool1.tile([128, 2, 512], F32, tag="sc")
        for sub in range(2):
            nc.tensor.matmul(
                sc[:, sub, b:512],
                kt_sb[:, hc, kis],
                qtz[:, 2 * hc + sub, 512 * qj + b : 512 * (qj + 1)],
                start=True,
                stop=True,
            )
        return sc

    def bp_norm(qj, hc):
        """Broadcast 1/den over partition halves + normalize one hc slot."""
        qsl = slice(512 * qj, 512 * (qj + 1))
        bp = pool2.tile([128, 512], F32, tag="ps")
        nc.tensor.matmul(
            bp,
            sel8[:, 128 * hc : 128 * (hc + 1)],
            rec8[:, qsl],
            start=True,
            stop=True,
        )
        nc.vector.tensor_mul(hout_sb[:, hc, qsl], hout_sb[:, hc, qsl], bp)

    def emit_O(qj, skip_norm=False):
        """Normalize (unless hoisted) + output projection + store."""
        qsl = slice(512 * qj, 512 * (qj + 1))
        if not skip_norm:
            with nc.allow_low_precision(reason="softmax reciprocal"):
                nc.vector.reciprocal(rec8[:, qsl], den8[:, qsl])
            for hc in range(4):
                bp_norm(qj, hc)
        for stile in range(4 * qj, 4 * qj + 4):
            out_sb = osb.tile([128, S], BF16, tag="out")
            for ej in range(2):
                op1 = pool2.tile([128, 512], F32, tag="ps")
                for hc in range(4):
                    nc.tensor.matmul(
                        op1,
                        hout_sb[:, hc, 128 * stile : 128 * (stile + 1)],
                        wo_sb[:, hc, 512 * ej : 512 * (ej + 1)],
                        start=(hc == 0),
                        stop=(hc == 3),
                    )
                esl = slice(512 * ej, 512 * (ej + 1))
                nc.scalar.copy(out_sb[:, esl], op1)
                nc.sync.dma_start(
                    out=t["out_p"][128 * stile : 128 * (stile + 1), esl],
                    in_=out_sb[:, esl],
                )

    steps = []
    for qj in range(2):
        kmax = 4 if qj == 0 else 8
        for hc in range(4):
            for ki in range(kmax):
                steps.append((qj, hc, ki, kmax))
    n_qj0 = 16

    def emit_exp(qj, hc, ki, sc):
        """exp (+ diagonal mask) for one step; returns the ee tile."""
        b = 128 * max(0, ki - 4 * qj)
        ee = epool.tile([128, 2, 512], BF16, tag="e", name="ee")
        nc.scalar.activation(
            ee[:, :, b:512], sc[:, :, b:512], EXP, scale=float(EXPSCALE)
        )
        if ki - 4 * qj >= 0:  # diagonal block: 0/1 mask, both heads at once
            nc.vector.tensor_mul(
                ee[:, :, b : b + 128],
                ee[:, :, b : b + 128],
                maskd[:, None, :].broadcast_to([128, 2, 128]),
            )
        return ee

    v_tpair(0)
    v_tpair(1)
    sc_next = emit_score(*steps[0][:3])
    avs = {}
    for i, (qj, hc, ki, kmax) in enumerate(steps):
        qsl = slice(512 * qj, 512 * (qj + 1))
        if ki == 0:
            av_e = pool2.tile([128, 512], F32, tag="ps")
            av_o = pool2.tile([128, 512], F32, tag="ps")
            avs[(qj, hc)] = (av_e, av_o)
        o_e, o_o = avs[(qj, hc)]
        sc = sc_next
        if i + 1 == n_qj0:  # qj=0 done being scored: V stiles 4-7 next
            v_tpair(2)
            v_tpair(3)
        if i + 1 < len(steps):  # cross-pair score lookahead
            sc_next = emit_score(*steps[i + 1][:3])
        b = 128 * max(0, ki - 4 * qj)
        ee = emit_exp(qj, hc, ki, sc)
        nc.tensor.matmul(
            o_e[0:65, b:512],
            v_sb[:, ki, 2 * hc, :],
            ee[:, 0, b:512],
            start=(ki == 0),
            stop=(ki == kmax - 1),
        )
        nc.tensor.matmul(
            o_o[0:65, b:512],
            v_sb[:, ki, 2 * hc + 1, :],
            ee[:, 1, b:512],
            start=(ki == 0),
            stop=(ki == kmax - 1),
        )
        if ki != kmax - 1:
            continue
        del avs[(qj, hc)]
        # denominator rows first: they unblock the reciprocal -> bp matmul
        # chain in phase O, so they must not queue behind the big copies
        de_t = xtr.tile([1, 512], F32, tag="de")
        do_t = xtr.tile([1, 512], F32, tag="do")
        nc.vector.tensor_copy(de_t, o_e[64:65, :])
        nc.vector.tensor_copy(do_t, o_o[64:65, :])
        nc.sync.dma_start(out=den8[2 * hc : 2 * hc + 1, qsl], in_=de_t)
        nc.sync.dma_start(out=den8[2 * hc + 1 : 2 * hc + 2, qsl], in_=do_t)
        # extract headout^T
        nc.vector.tensor_copy(hout_sb[0:64, hc, qsl], o_e[0:64, :])
        otmp = xtr.tile([64, 512], BF16, tag="otmp")
        nc.vector.tensor_copy(otmp, o_o[0:64, :])
        nc.sync.dma_start(out=hout_sb[64:128, hc, qsl], in_=otmp)
        # hoisted qj0 normalize, spread over qj1 pair boundaries (these
        # run inside the pair-end block, so i is always a boundary): the
        # recip's den8 inputs are 8 steps old (no DVE stall), each bp
        # pair lands on ring slots freed two pairs earlier, and phase O
        # starts with qj0 already normalized.
        if i == 23:
            with nc.allow_low_precision(reason="softmax reciprocal"):
                nc.vector.reciprocal(rec8[:, 0:512], den8[:, 0:512])
        elif i == 31:
            bp_norm(0, 0)
            bp_norm(0, 1)
        elif i == 39:
            bp_norm(0, 2)
            bp_norm(0, 3)

    # next-iteration activation refresh on the SP ring: issues after the
    # den8 gathers, ahead of the out_p stores (which may lag into the next
    # iteration's P phase -- they are only read at run end)
    for which in ("q", "k", "v"):
        xdr = t["x" + which + "_t"].rearrange("(n p) s -> p n s", p=128)
        for half in range(2):
            hs = slice(512 * half, 512 * (half + 1))
            nc.sync.dma_start(
                out=tl["x8" + which][:, :, hs], in_=xdr[:, :, hs]
            )

    emit_O(0, skip_norm=True)
    emit_O(1)


def _build(repeat=1):
    nc = bacc.Bacc()
    t = {}
    for name in ("xq_t", "xk_t", "xv_t"):
        t[name] = nc.dram_tensor(name, [D, S], BF16, kind="ExternalInput")
    for name in ("wq_t", "wk_t", "wv_t"):
        t[name] = nc.dram_tensor(name, [128, 8 * 512], BF16, kind="ExternalInput")
    t["wo_s"] = nc.dram_tensor("wo_s", [128, 4 * D], BF16, kind="ExternalInput")
    t["maskd"] = nc.dram_tensor("maskd", [128, 128], BF16, kind="ExternalInput")
    t["sel8"] = nc.dram_tensor("sel8", [8, 512], F32R, kind="ExternalInput")
    t["out_p"] = nc.dram_tensor("out_p", [S, D], BF16, kind="ExternalOutput")

    with tile.TileContext(nc) as tc:
        with ExitStack() as ctx:
            pl, tl = _alloc_persistent(nc, tc, ctx, t)
            n_loop, rem = divmod(repeat, UNROLL)
            if n_loop > 0:
                with tc.For_i(0, n_loop, 1):
                    for _ in range(UNROLL):
                        _emit_iter(nc, tc, t, tl, pl)
            for _ in range(rem):
                _emit_iter(nc, tc, t, tl, pl)
    nc.compile()
    return nc


_CACHE = {}


def _get(repeat=1):
    if repeat not in _CACHE:
        _CACHE[repeat] = _build(repeat)
    return _CACHE[repeat]


def _host_prep(query, key, value, mask, Wq, Wk, Wv, Wo):
    """Build the per-core in_maps. Returns None if mask isn't causal tril."""
    m = np.asarray(mask)[0, 0]
    if not np.array_equal(m, np.tril(np.ones((S, S), m.dtype))):
        return None

    bf = ml_dtypes.bfloat16

    # diagonal-block mask (same for every diagonal tile under causal tril)
    maskd = m[0:128, 0:128].T.astype(bf)

    sel8 = np.zeros((8, 512), np.float32)
    for hc in range(4):
        sel8[2 * hc, 128 * hc : 128 * hc + 64] = 1.0
        sel8[2 * hc + 1, 128 * hc + 64 : 128 * hc + 128] = 1.0

    def ileave(a):  # [R, C] -> [128, (R//128)*C]: chunk-c data contiguous per p
        R, C = a.shape
        return np.ascontiguousarray(
            a.reshape(R // 128, 128, C).transpose(1, 0, 2).reshape(128, -1)
        )

    in_maps = []
    for c in range(N_CORES):
        b, g = c // 2, c % 2
        gsl = slice(512 * g, 512 * (g + 1))
        in_maps.append(
            {
                "xq_t": np.ascontiguousarray(query[b].T.astype(bf)),
                "xk_t": np.ascontiguousarray(key[b].T.astype(bf)),
                "xv_t": np.ascontiguousarray(value[b].T.astype(bf)),
                "wq_t": ileave(Wq[gsl, :].T.astype(bf)),
                "wk_t": ileave(Wk[gsl, :].T.astype(bf)),
                "wv_t": ileave(Wv[gsl, :].T.astype(bf)),
                "wo_s": ileave(Wo[:, gsl].T.astype(bf)),
                "maskd": maskd,
                "sel8": sel8,
            }
        )
    return in_maps


def _gather(results, bo, B):
    out = np.empty((B, S, D), np.float32)
    for b in range(B):
        out[b] = (
            results[2 * b]["out_p"].astype(np.float32)
            + results[2 * b + 1]["out_p"].astype(np.float32)
            + np.asarray(bo)[None, :]
        )
    return out


def _reference_fallback(query, key, value, mask, Wq, Wk, Wv, Wo, bo):
    B = query.shape[0]
    H = 16
    dk = D // H
    q = np.asarray(query, np.float32)
    k = np.asarray(key, np.float32)
    v = np.asarray(value, np.float32)

    def proj(x, W):
        return (x @ W.T).reshape(B, S, H, dk).transpose(0, 2, 1, 3)

    Q, K, V = proj(q, Wq), proj(k, Wk), proj(v, Wv)
    sc = np.einsum("bhqd,bhkd->bhqk", Q, K) / np.sqrt(np.float32(dk))
    sc = np.where(np.asarray(mask) == 0, np.float32(-1e9), sc)
    sc = sc - sc.max(axis=-1, keepdims=True)
    a = np.exp(sc)
    a = a / a.sum(axis=-1, keepdims=True)
    o = np.einsum("bhqk,bhkd->bhqd", a, V).transpose(0, 2, 1, 3).reshape(B, S, D)
    return (o @ np.asarray(Wo).T + np.asarray(bo)).astype(np.float32)


def kernel(query, key, value, mask, Wq, Wk, Wv, Wo, bo):
    query = np.asarray(query, np.float32)
    key = np.asarray(key, np.float32)
    value = np.asarray(value, np.float32)
    Wq, Wk, Wv, Wo = (np.asarray(w, np.float32) for w in (Wq, Wk, Wv, Wo))
    in_maps = _host_prep(query, key, value, mask, Wq, Wk, Wv, Wo)
    if in_maps is None:  # non-causal mask: host fallback
        return _reference_fallback(query, key, value, mask, Wq, Wk, Wv, Wo, bo)
    nc = _get(1)
    res = run_bass_kernel_spmd(nc, in_maps, list(range(N_CORES)))
    return _gather(res.results, bo, query.shape[0])


def run_spmd(in_maps, repeat=1):
    """For test.py: run prebuilt kernel, return BassKernelResults."""
    nc = _get(repeat)
    return run_bass_kernel_spmd(nc, in_maps, list(range(N_CORES)))


def host_prep(*args, **kw):
    return _host_prep(*args, **kw)


def gather(results, bo, B=4):
    return _gather(results, bo, B)

